# revision 12
# baseline (speedup 1.0000x reference)
import sys

sys.path.insert(0, "/opt/trn_rl_repo")

import numpy as np

import concourse.bass as bass  # noqa: F401
import concourse.tile as tile
from concourse import bacc, mybir
from concourse.bass_utils import run_bass_kernel_spmd

# Multi-head self-attention, Q == K (shared qk projection).
# N=4096 tokens, D=1024 model dim, H=16 heads, DK=64 head dim.
# Head-tensor-parallel over 8 cores: core c owns heads 2c, 2c+1.
# Per core: W shard [1024, 256] = [qk cols c*128:(c+1)*128 | v cols same],
# output shard [4096, 128] = global out columns c*128:(c+1)*128.
#
# Per-core algorithm (heads h0, h1 local):
#   QT[j, n] = (X @ Wq)[n, j]^T   j = h*64+dk   (computed directly: W^T X^T)
#   V_sb[m_local, mb*128 + j]     natural-orientation V blocks
#   S symmetric (Q==K): compute E-tiles E[m_local, n] = exp(S[m, n]/8) per
#   (m-block, n-quarter); row-sums via activation accum_out give Z;
#   AV: outT[j, n] += V_block^T @ E accumulated over all 32 m-blocks;
#   epilogue: transpose outT back to [n, j] and scale rows by 1/Z[n].

N = 4096
D = 1024
H = 16
DK = 64
NCORES = 8
HPC = 2                # heads per core
WCOLS = 256            # 128 qk cols + 128 v cols
KC = 8                 # contraction chunks of 128 over D
MC = 8                 # m chunks of 512
MB = 32                # m blocks of 128
NQ = 4                 # n quarters of 1024

F32 = mybir.dt.float32
F32R = mybir.dt.float32r
EXP = mybir.ActivationFunctionType.Exp


def _r(ap):
    return ap.bitcast(F32R)


def _build(repeat=1, num_devices=NCORES):
    nc = bacc.Bacc("TRN2", target_bir_lowering=False, debug=False,
                   num_devices=num_devices)
    X_d = nc.dram_tensor("X", [N, D], F32, kind="ExternalInput")
    W_d = nc.dram_tensor("W", [D, WCOLS], F32, kind="ExternalInput")
    O_d = nc.dram_tensor("OUT", [N, HPC * DK], F32, kind="ExternalOutput")
    ident_d = nc.inline_tensor(np.eye(128, dtype=np.float32), name="ident")

    with tile.TileContext(nc) as tc:
        with tc.tile_pool(name="persist", bufs=1) as persist:
            ident = persist.tile([128, 128], F32)
            nc.sync.dma_start(ident[:], ident_d.ap()[:])
            W_raw = persist.tile([128, KC, WCOLS], F32)
            for kc in range(KC):
                nc.sync.dma_start(W_raw[:, kc, :],
                                  W_d.ap()[kc * 128:(kc + 1) * 128, :])
            W_sb = persist.tile([128, KC, WCOLS], F32)
            nc.vector.tensor_copy(W_sb[:].bitcast(F32R), W_raw[:])
            QT = persist.tile([128, N], F32)      # [j, n] q^T, j=h*64+dk
            V_sb = persist.tile([128, N], F32)    # [m_local, mb*128+j]
            OT0_sb = persist.tile([64, N], F32)   # [dv, n] head0 out^T
            OT1_sb = persist.tile([64, N], F32)   # [dv, n] head1 out^T
            Z_all = persist.tile([128, MB * HPC, NQ], F32)
            Zsum = persist.tile([128, MB * HPC], F32)
            R_sb = persist.tile([128, MB * HPC], F32)

            for _ in range(repeat):
                # ---------- phase 1: projection ----------
                with tc.tile_pool(name="xr", bufs=2) as xr_pool, \
                     tc.tile_pool(name="xt", bufs=1) as xt_pool, \
                     tc.tile_pool(name="vt", bufs=2) as vt_pool, \
                     tc.tile_pool(name="tp", bufs=2, space="PSUM") as tp_pool, \
                     tc.tile_pool(name="pq", bufs=1, space="PSUM") as pq_pool, \
                     tc.tile_pool(name="pv", bufs=1, space="PSUM") as pv_pool, \
                     tc.tile_pool(name="vp", bufs=2, space="PSUM") as vp_pool:
                    for mc in range(MC):
                        XT = xt_pool.tile([128, KC, 512], F32)
                        for rb in range(4):
                            xr = xr_pool.tile([128, D], F32)
                            row0 = mc * 512 + rb * 128
                            nc.sync.dma_start(xr[:],
                                              X_d.ap()[row0:row0 + 128, :])
                            tp = tp_pool.tile([128, KC, 128], F32)
                            for kc in range(KC):
                                nc.tensor.transpose(
                                    tp[:, kc, :],
                                    xr[:, kc * 128:(kc + 1) * 128],
                                    ident[:])
                            nc.vector.tensor_copy(
                                XT[:, :, rb * 128:(rb + 1) * 128]
                                .bitcast(F32R), tp[:])
                        pq = pq_pool.tile([128, 512], F32)
                        pv = pv_pool.tile([128, 512], F32)
                        for kc in range(KC):
                            nc.tensor.matmul(pq[:], _r(W_sb[:, kc, 0:128]),
                                             _r(XT[:, kc, :]),
                                             start=(kc == 0),
                                             stop=(kc == KC - 1))
                        for kc in range(KC):
                            nc.tensor.matmul(pv[:], _r(W_sb[:, kc, 128:256]),
                                             _r(XT[:, kc, :]),
                                             start=(kc == 0),
                                             stop=(kc == KC - 1))
                        nc.vector.tensor_copy(
                            QT[:, mc * 512:(mc + 1) * 512].bitcast(F32R),
                            pq[:])
                        vt = vt_pool.tile([128, 512], F32)
                        nc.vector.tensor_copy(vt[:], pv[:])
                        vp = vp_pool.tile([128, 512], F32)
                        for t in range(4):
                            nc.tensor.transpose(vp[:, t * 128:(t + 1) * 128],
                                                vt[:, t * 128:(t + 1) * 128],
                                                ident[:])
                        nc.vector.tensor_copy(
                            V_sb[:, mc * 512:(mc + 1) * 512].bitcast(F32R),
                            vp[:])

                # ---------- phase 2: attention ----------
                with tc.tile_pool(name="sps", bufs=1, space="PSUM") as s_pool, \
                     tc.tile_pool(name="ops", bufs=1, space="PSUM") as o_pool, \
                     tc.tile_pool(name="eb", bufs=4) as e_pool:

                    def emit_av(E0, E1, outT0, outT1, q, mb):
                        for E, outT, base in ((E0, outT0, mb * 128),
                                              (E1, outT1, mb * 128 + 64)):
                            for j in range(2):
                                nc.tensor.matmul(
                                    outT[:, j * 512:(j + 1) * 512],
                                    _r(V_sb[:, base:base + 64]),
                                    _r(E[:, j * 512:(j + 1) * 512]),
                                    start=(mb == 0), stop=(mb == MB - 1),
                                    skip_group_check=True)
                        if mb == MB - 1:
                            nc.vector.tensor_copy(
                                OT0_sb[:, q * 1024:(q + 1) * 1024], outT0[:])
                            nc.vector.tensor_copy(
                                OT1_sb[:, q * 1024:(q + 1) * 1024], outT1[:])

                    pend = None
                    for q in range(NQ):
                        outT0 = o_pool.tile([64, 1024], F32)
                        outT1 = o_pool.tile([64, 1024], F32)
                        for mb in range(MB):
                            s0 = s_pool.tile([128, 1024], F32)
                            s1 = s_pool.tile([128, 1024], F32)
                            for j in range(2):
                                nc.tensor.matmul(
                                    s0[:, j * 512:(j + 1) * 512],
                                    _r(QT[0:64, mb * 128:(mb + 1) * 128]),
                                    _r(QT[0:64, q * 1024 + j * 512:
                                           q * 1024 + (j + 1) * 512]),
                                    start=True, stop=True,
                                    skip_group_check=True)
                                nc.tensor.matmul(
                                    s1[:, j * 512:(j + 1) * 512],
                                    _r(QT[64:128, mb * 128:(mb + 1) * 128]),
                                    _r(QT[64:128, q * 1024 + j * 512:
                                           q * 1024 + (j + 1) * 512]),
                                    start=True, stop=True,
                                    skip_group_check=True)
                            E0 = e_pool.tile([128, 1024], F32)
                            E1 = e_pool.tile([128, 1024], F32)
                            nc.scalar.activation(
                                E0[:].bitcast(F32R), s0[:], EXP, scale=0.125,
                                accum_out=Z_all[:, mb * HPC + 0, q:q + 1])
                            nc.scalar.activation(
                                E1[:].bitcast(F32R), s1[:], EXP, scale=0.125,
                                accum_out=Z_all[:, mb * HPC + 1, q:q + 1])
                            if pend is not None:
                                emit_av(*pend)
                            pend = (E0, E1, outT0, outT1, q, mb)
                    emit_av(*pend)

                # ---------- Z finalize + epilogue ----------
                nc.vector.tensor_reduce(Zsum[:], Z_all[:],
                                        axis=mybir.AxisListType.X,
                                        op=mybir.AluOpType.add)
                nc.vector.reciprocal(R_sb[:], Zsum[:])

                with tc.tile_pool(name="ep", bufs=2, space="PSUM") as ep_pool, \
                     tc.tile_pool(name="ob", bufs=3) as ob_pool:
                    for nb in range(MB):
                        tpo = ep_pool.tile([128, 128], F32)
                        nc.tensor.transpose(
                            tpo[:, 0:64], OT0_sb[:, nb * 128:(nb + 1) * 128],
                            ident[0:64, 0:64])
                        nc.tensor.transpose(
                            tpo[:, 64:128], OT1_sb[:, nb * 128:(nb + 1) * 128],
                            ident[0:64, 0:64])
                        osb = ob_pool.tile([128, 128], F32)
                        for h in range(HPC):
                            idx = nb * HPC + h
                            nc.vector.tensor_scalar_mul(
                                osb[:, h * 64:(h + 1) * 64],
                                tpo[:, h * 64:(h + 1) * 64],
                                R_sb[:, idx:idx + 1])
                        nc.sync.dma_start(
                            O_d.ap()[nb * 128:(nb + 1) * 128, :], osb[:])

    nc.compile()
    return nc


_CACHE = {}


def kernel(X, Wqkv):
    X = np.ascontiguousarray(np.asarray(X, dtype=np.float32))
    Wqkv = np.asarray(Wqkv, dtype=np.float32)
    prog = _CACHE.get("prog")
    if prog is None:
        prog = _build()
        _CACHE["prog"] = prog
    in_maps = []
    for c in range(NCORES):
        Wsh = np.ascontiguousarray(np.concatenate(
            [Wqkv[:, c * 128:(c + 1) * 128],
             Wqkv[:, D + c * 128:D + (c + 1) * 128]], axis=1))
        in_maps.append({"X": X, "W": Wsh})
    res = run_bass_kernel_spmd(prog, in_maps, core_ids=list(range(NCORES)))
    return np.concatenate([res.results[c]["OUT"] for c in range(NCORES)],
                          axis=1)
